# revision 54
# baseline (speedup 1.0000x reference)
"""Trainium2 Bass kernel: multi-head attention (B=2, T=2048, E=1024, H=8, D=512),
bias-free QKV/O projections + RoPE + causal softmax.

Sharding: head-parallel across 8 NeuronCores. Core h computes head h fully:
  qT/kT = RoPE(Wq_h @ x.T), v = x @ Wv_h.T         (projection phase)
  scoresT[k,q] = kT.T @ qT   (per 512-wide q tile, causal-skipped k chunks;
    diagonal chunk r also skips its fully-masked first 128r columns)
  causal mask applied as a 5th accumulated matmul: (-C*[j<=kk]).T @ [j>qq-128r]
  gives -C*max(0, kk+128r-qq) in PSUM, so exp sees -inf off the triangle and
  the score->exp chain never leaves the PE/ACT engines.
  probsT = exp(scale*scoresT)                      (no max-subtraction: |s|<=9)
  attnT[d,q] = v.T @ probsT (unnormalized!); the softmax denominator is
  accumulated on the idle Pool engine (acc += exp chunk), reduced to a
  per-TOKEN column by 4 tiny transposed matmuls (acc_slice.T @ ones), and a
  [128,16] reciprocal; since 1/rowsum is a per-token scalar it commutes with
  o_proj and is fused into the o_proj PSUM evacuation as an ACT scale.
  out_h = attnT.T @ Wo_h.T * inv                   (partial o_proj, [4096,1024])
Host sums the 8 partial outputs (equivalent to the all-reduce after o_proj).

Matmuls run fp32r except PV + o_proj which run bf16 (probs/v/attn/Wo are
bf16: probs in [0,1] and attn magnitudes are tolerant; PSUM accumulates fp32).
fp32r moving dims stay >= 256 (ISA restriction; narrower runs 4 cyc/row).
PSUM is managed as 8 explicitly-tagged banks so phase-to-phase reuse pairs
earliest-freed banks with earliest-needed, keeping the PE stream gap-free:
startup DMAs are split per 256KB chunk with e-outer matmul ordering so the PE
chases the stream from ~10us; batch 1's first x tile is prefetched during
batch 0's attention and its v projection fills the PE while batch 0's final
rowsum chain drains; the previous q tile's o_proj is emitted one token tile
per chunk so its PSUM evacuations interleave with the exps on ACT.
Measured ~394-402us (baseline 446us), rel err ~3.2e-3.
"""
from contextlib import ExitStack

import numpy as np

B, T, E, H, D = 2, 2048, 1024, 8, 512
NTOK = B * T
SCALE = float(1.0 / np.sqrt(D))
NEG = -1.0e30
ROPE_BASE = 10000.0

PROFILE = False          # set True (e.g. from test.py) to trace core 0
LAST_RESULTS = None      # BassKernelResults of the last run when PROFILE

_CACHE = {}


def _build():
    import concourse.tile as tile
    from concourse import bacc, mybir

    f32 = mybir.dt.float32
    f32r = mybir.dt.float32r
    bf16 = mybir.dt.bfloat16
    AF = mybir.ActivationFunctionType

    nc = bacc.Bacc("TRN2", target_bir_lowering=False, debug=False,
                   enable_asserts=False, num_devices=8)
    xT_d = nc.dram_tensor("xT", [E, NTOK], f32r, kind="ExternalInput").ap()
    wqT_d = nc.dram_tensor("wqT", [E, D], f32r, kind="ExternalInput").ap()
    wkT_d = nc.dram_tensor("wkT", [E, D], f32r, kind="ExternalInput").ap()
    wvT_d = nc.dram_tensor("wvT", [E, D], f32r, kind="ExternalInput").ap()
    woT_d = nc.dram_tensor("woT", [D, E], bf16, kind="ExternalInput").ap()
    cos_d = nc.dram_tensor("cosdt", [D // 2, T], f32, kind="ExternalInput").ap()
    sin_d = nc.dram_tensor("sindt", [D // 2, T], f32, kind="ExternalInput").ap()
    a0_d = nc.dram_tensor("a0t", [128, 128], bf16, kind="ExternalInput").ap()
    bm_d = nc.dram_tensor("bmt", [128, 4, 512], bf16, kind="ExternalInput").ap()
    out_d = nc.dram_tensor("out", [NTOK, E], f32, kind="ExternalOutput").ap()

    xT_r = xT_d.rearrange("(eo p) t -> p eo t", p=128)     # [128, 8, 4096]
    cos_r = cos_d.rearrange("(fo p) t -> p fo t", p=128)   # [128, 2, 2048]
    sin_r = sin_d.rearrange("(fo p) t -> p fo t", p=128)
    wq_v = wqT_d.rearrange("(eo p) d -> p eo d", p=128)
    wk_v = wkT_d.rearrange("(eo p) d -> p eo d", p=128)
    wv_v = wvT_d.rearrange("(eo p) d -> p eo d", p=128)

    with tile.TileContext(nc) as tc, ExitStack() as top:
        # ---- all pools top-level: no mid-kernel pool boundaries ----
        wp = top.enter_context(tc.tile_pool(name="wp", bufs=1))
        qkvp = top.enter_context(tc.tile_pool(name="qkvp", bufs=1))
        xp = top.enter_context(tc.tile_pool(name="xp", bufs=2))
        csp = top.enter_context(tc.tile_pool(name="csp", bufs=1))
        scrp = top.enter_context(tc.tile_pool(name="scrp", bufs=2))
        epp = top.enter_context(tc.tile_pool(name="epp", bufs=4))
        atp = top.enter_context(tc.tile_pool(name="atp", bufs=1))
        accp = top.enter_context(tc.tile_pool(name="accp", bufs=1))
        ivp = top.enter_context(tc.tile_pool(name="ivp", bufs=1))
        pbp = top.enter_context(tc.tile_pool(name="pbp", bufs=1, space="PSUM"))

        def PB(k):
            return pbp.tile([128, 512], f32, tag=f"B{k}", name=f"B{k}")

        wq_t = wp.tile([128, 8, D], f32r, tag="wq", name="wq")
        wk_t = wp.tile([128, 8, D], f32r, tag="wk", name="wk")
        wv_t = wp.tile([128, 8, D], f32r, tag="wv", name="wv")
        wv = [wv_t[:, e] for e in range(8)]
        a0_t = wp.tile([128, 128], bf16, tag="a0", name="a0")
        bm_t = wp.tile([128, 4, 512], bf16, tag="bm", name="bm")
        ones = wp.tile([128, 128], f32r, tag="ones", name="ones")
        onesf = wp.tile([128, 4], f32, tag="onesf", name="onesf")
        expre = wp.tile([128, 1], f32, tag="expre", name="expre")
        wo_t = wp.tile([128, 4, E], bf16, tag="wo", name="wo")
        wo = [wo_t[:, d] for d in range(4)]

        qT = [qkvp.tile([128, T], f32r, tag=f"qT{d}", name=f"qT{d}") for d in range(4)]
        kT = [qkvp.tile([128, T], f32r, tag=f"kT{d}", name=f"kT{d}") for d in range(4)]
        vv = [qkvp.tile([128, D], bf16, tag=f"v{t}", name=f"v{t}") for t in range(16)]

        def rope(dstT, i, j, fo, pi, pj, cs, sn, s0):
            c_, s_ = cs[:, fo], sn[:, fo]
            sA = scrp.tile([128, 1024], f32, tag="scr", name="scr")
            sB = scrp.tile([128, 1024], f32, tag="scr", name="scr")
            t0, t1 = sA[:, 0:512], sA[:, 512:1024]
            t2, t3 = sB[:, 0:512], sB[:, 512:1024]
            # both pi reads first, then both pj reads: each PSUM bank is
            # released after 2 DVE ops instead of 4/5, easing the 6-bank
            # qk rotation that the PE production rate leans on
            nc.vector.tensor_mul(t0[:], pi[:], c_)
            nc.vector.tensor_mul(t2[:], pi[:], s_)
            nc.vector.tensor_mul(t1[:], pj[:], s_)
            nc.vector.tensor_mul(t3[:], pj[:], c_)
            nc.vector.tensor_sub(dstT[i][:, s0:s0 + 512], t0[:], t1[:])
            nc.vector.tensor_add(dstT[j][:, s0:s0 + 512], t2[:], t3[:])

        def emit_v(tt, xt, banks=(4, 5)):
            for t4 in range(4):
                ps_t = PB(banks[t4 % len(banks)])
                for e in range(8):
                    nc.tensor.matmul(
                        ps_t[:],
                        xt[:, e, t4 * 128:(t4 + 1) * 128],
                        wv[e][:],
                        start=(e == 0), stop=(e == 7))
                nc.scalar.copy(vv[tt * 4 + t4][:], ps_t[:])

        def emit_v_eouter(tt, xt, banks=(0, 1, 2, 3)):
            vps = [PB(banks[t4]) for t4 in range(4)]
            for e in range(8):
                for t4 in range(4):
                    nc.tensor.matmul(
                        vps[t4][:],
                        xt[:, e, t4 * 128:(t4 + 1) * 128],
                        wv[e][:],
                        start=(e == 0), stop=(e == 7))
            for t4 in range(4):
                nc.scalar.copy(vv[tt * 4 + t4][:], vps[t4][:])

        def emit_qk(s0, xt, cs, sn):
            rot = (6, 7, 0, 1, 2, 3)
            ri = 0
            for w_t, dstT in ((wq_t, qT), (wk_t, kT)):
                for i, j, fo in ((0, 2, 0), (1, 3, 1)):
                    ps2 = []
                    for dc in (i, j):
                        ps_t = PB(rot[ri % 6])
                        ri += 1
                        for e in range(8):
                            nc.tensor.matmul(
                                ps_t[:],
                                w_t[:, e, dc * 128:(dc + 1) * 128],
                                xt[:, e],
                                start=(e == 0), stop=(e == 7))
                        ps2.append(ps_t)
                    rope(dstT, i, j, fo, ps2[0], ps2[1], cs, sn, s0)

        def emit_qk_eouter(s0, xt, cs, sn):
            # all 4 dc of one projection accumulate e-outer so each matmul
            # needs only one freshly-DMA'd 256KB chunk pair
            for w_t, dstT, b0k in ((wq_t, qT, 4), (wk_t, kT, 0)):
                ps = {dc: PB(b0k + di) for di, dc in enumerate((0, 2, 1, 3))}
                for e in range(8):
                    for dc in (0, 2, 1, 3):
                        nc.tensor.matmul(
                            ps[dc][:],
                            w_t[:, e, dc * 128:(dc + 1) * 128],
                            xt[:, e],
                            start=(e == 0), stop=(e == 7))
                rope(dstT, 0, 2, 0, ps[0], ps[2], cs, sn, s0)
                rope(dstT, 1, 3, 1, ps[1], ps[3], cs, sn, s0)

        # =========== batch 0 projection ===========
        cs0 = {}
        for tt in range(4):
            g0 = tt * 512
            s0 = tt * 512
            xt = xp.tile([128, 8, 512], f32r, tag="xt", name="xt")
            cs = csp.tile([128, 2, 512], f32, tag="cs", name="cs")
            sn = csp.tile([128, 2, 512], f32, tag="sn", name="sn")
            if tt == 0:
                # need-ordered per-chunk DMAs: the PE chases the stream
                for e in range(8):
                    nc.sync.dma_start(xt[:, e], xT_r[:, e, g0:g0 + 512])
                    nc.sync.dma_start(wv_t[:, e], wv_v[:, e])
                for e in range(8):
                    nc.sync.dma_start(wq_t[:, e], wq_v[:, e])
                nc.sync.dma_start(cs[:], cos_r[:, :, s0:s0 + 512])
                nc.sync.dma_start(sn[:], sin_r[:, :, s0:s0 + 512])
                for e in range(8):
                    nc.sync.dma_start(wk_t[:, e], wk_v[:, e])
                # warmup: ramp the PE clock while the first DMAs stream in
                # (memset cannot target f32r: set f32 then cast-copy)
                onef = scrp.tile([128, 1024], f32, tag="scr", name="scr")
                nc.vector.memset(onef[:, :128], 1.0)
                nc.vector.tensor_copy(ones[:], onef[:, :128])
                nc.vector.memset(onesf[:], 1.0)
                warm_ps = PB(4)
                for w in range(14):
                    nc.tensor.matmul(warm_ps[:, :128], ones[:], ones[:],
                                     start=(w == 0), stop=(w == 13))
                nc.scalar.activation(expre[:], warm_ps[:, :1], AF.Exp,
                                     scale=0.001)
                nc.vector.tensor_copy(expre[:], expre[:])
                emit_v_eouter(tt, xt)
                emit_qk_eouter(s0, xt, cs, sn)
            elif tt == 1:
                for e in range(8):
                    nc.sync.dma_start(xt[:, e], xT_r[:, e, g0:g0 + 512])
                nc.sync.dma_start(cs[:], cos_r[:, :, s0:s0 + 512])
                nc.sync.dma_start(sn[:], sin_r[:, :, s0:s0 + 512])
                emit_v_eouter(tt, xt, banks=(4, 5, 6, 7))
                emit_qk(s0, xt, cs, sn)
            else:
                nc.sync.dma_start(xt[:], xT_r[:, :, g0:g0 + 512])
                nc.sync.dma_start(cs[:], cos_r[:, :, s0:s0 + 512])
                nc.sync.dma_start(sn[:], sin_r[:, :, s0:s0 + 512])
                if tt == 2:
                    emit_v(tt, xt)
                    emit_qk(s0, xt, cs, sn)
                else:
                    emit_qk(s0, xt, cs, sn)
                    emit_v(tt, xt)

        # prefetch batch 1's first x tile + rope tables; they transfer
        # during batch 0's attention so its projection starts stall-free
        xt_pre = xp.tile([128, 8, 512], f32r, tag="xt", name="xt")
        cs_pre = csp.tile([128, 2, 512], f32, tag="cs", name="cs")
        sn_pre = csp.tile([128, 2, 512], f32, tag="sn", name="sn")

        # =========== attention (both batches) ===========
        def attn_phase(b, tail_filler=None):
            tok0 = b * T
            SC = (0, 1, 2)
            ATB = (3, 4, 6, 7)
            if b == 0:
                nc.sync.dma_start(a0_t[:], a0_d)
                nc.sync.dma_start(bm_t[:], bm_d)
                nc.sync.dma_start(
                    wo_t[:], woT_d.rearrange("(do p) e -> p do e", p=128))
                nc.sync.dma_start(xt_pre[:], xT_r[:, :, T:T + 512])
                nc.sync.dma_start(cs_pre[:], cos_r[:, :, 0:512])
                nc.sync.dma_start(sn_pre[:], sin_r[:, :, 0:512])

            sci = [0]

            def emit_oproj_t4(n, t4, split_dma=False):
                q0 = n * 512
                r0 = tok0 + q0 + t4 * 128
                ob = scrp.tile([128, 1024], f32, tag="scr", name="scr")
                for et in range(2):
                    op_ps = PB(SC[sci[0] % 3])
                    sci[0] += 1
                    for dc in range(4):
                        nc.tensor.matmul(
                            op_ps[:],
                            at_sb[n % 2][dc][:, t4 * 128:(t4 + 1) * 128],
                            wo[dc][:, et * 512:(et + 1) * 512],
                            start=(dc == 0), stop=(dc == 3))
                    # attn was left unnormalized; the softmax 1/rowsum is
                    # a per-token scalar, so it commutes with o_proj and
                    # fuses into this evacuation copy for free
                    nc.scalar.activation(
                        ob[:, et * 512:(et + 1) * 512], op_ps[:],
                        AF.Copy, scale=inv_col[n % 2][:, t4 * 4:t4 * 4 + 1])
                    if split_dma:
                        nc.sync.dma_start(
                            out_d[r0:r0 + 128, et * 512:(et + 1) * 512],
                            ob[:, et * 512:(et + 1) * 512])
                if not split_dma:
                    nc.sync.dma_start(out_d[r0:r0 + 128, :], ob[:])

            at_sb = {0: None, 1: None}
            inv_col = {0: None, 1: None}
            for n in range(4):
                q0 = n * 512
                nch = 4 * n + 4
                attn_ps = [PB(ATB[d]) for d in range(4)]
                acc = accp.tile([128, 512], f32, tag="acc", name="acc")

                def emit_pv(pex, pc, nch=nch, attn_ps=attn_ps, n=n):
                    # columns [0, 128r) of a diagonal chunk are fully masked
                    # (exp == 0): skip them
                    s = max(0, (pc - 4 * n)) * 128
                    last = pc == nch - 1
                    for dc in range(4):
                        nc.tensor.matmul(
                            attn_ps[dc][:, s:512],
                            vv[pc][:, dc * 128:(dc + 1) * 128], pex[:, s:512],
                            start=(pc == 0), stop=last,
                            skip_group_check=(s > 0))

                pending = []
                for c in range(nch):
                    diag = c - 4 * n
                    sc_ps = PB(SC[sci[0] % 3])
                    sci[0] += 1
                    # diagonal chunk r: columns [0, 128r) are fully masked.
                    # scores (f32r) must stay >=256 wide; PV/exp/rowsum use
                    # the exact boundary s_pv = 128r
                    s_pv = max(0, diag) * 128
                    s_sc = min(s_pv, 256)
                    for dc in range(4):
                        nc.tensor.matmul(
                            sc_ps[:, s_sc:512],
                            kT[dc][:, c * 128:(c + 1) * 128],
                            qT[dc][:, q0 + s_sc:q0 + 512],
                            start=(dc == 0),
                            stop=(dc == 3))
                    if diag >= 0:
                        # causal mask as a 5th (narrow) accumulated matmul:
                        # -C * max(0, kk + 128*diag - qq) lands in PSUM; only
                        # the computed-and-maskable span [s_sc, 128(r+1)) or
                        # [s_sc, 512) for r == 3 needs it
                        w = 512 if diag == 3 else 128 * (diag + 1)
                        nc.tensor.matmul(
                            sc_ps[:, s_sc:w], a0_t[:], bm_t[:, diag, s_sc:w],
                            start=False, stop=True, skip_group_check=True)
                    ex = epp.tile([128, 512], bf16, tag="ex", name="ex")
                    nc.scalar.activation(
                        ex[:, s_sc:512], sc_ps[:, s_sc:512], AF.Exp, scale=SCALE)
                    # rowsum accumulates on the (otherwise idle) Pool engine
                    if c == 0:
                        nc.gpsimd.tensor_copy(acc[:], ex[:])
                    else:
                        nc.gpsimd.tensor_add(
                            acc[:, s_pv:512], acc[:, s_pv:512], ex[:, s_pv:512])
                    pending.append((ex, c))
                    if len(pending) > 3:
                        emit_pv(*pending.pop(0))
                    # previous group's o_proj, one token tile at a time so
                    # its PSUM evacuations interleave with the exps on ACT
                    if 2 <= c <= 5 and n > 0:
                        emit_oproj_t4(n - 1, c - 2)
                for pex, pc in pending:
                    emit_pv(pex, pc)
                # rowsum as a per-token COLUMN: 4 tiny transposed matmuls
                # (acc_slice.T @ ones[:, :1]); the reciprocal then runs on
                # just [128, 4] elements instead of [128, 512]
                rs_ps = PB(5)
                for t4 in range(4):
                    nc.tensor.matmul(
                        rs_ps[:, t4 * 4:(t4 + 1) * 4],
                        acc[:, t4 * 128:(t4 + 1) * 128], onesf[:, 0:4],
                        start=True, stop=True)
                inv_col[n % 2] = ivp.tile(
                    [128, 16], f32, tag=f"inv{n % 2}", name=f"inv{n % 2}")
                nc.vector.reciprocal(inv_col[n % 2][:], rs_ps[:, 0:16])
                # evacuate UNNORMALIZED via fast ACT copies: frees the PSUM
                # banks without waiting on any DVE chain
                at_sb[n % 2] = [
                    atp.tile([128, 512], bf16, tag=f"at{n % 2}_{dc}",
                             name=f"at{n % 2}_{dc}")
                    for dc in range(4)]
                if n == 3:
                    if tail_filler is not None:
                        tail_filler()
                    # final o_proj: evacuate each token tile's attn slice and
                    # immediately emit its o_proj so ACT copies, matmuls and
                    # output DMAs pipeline tightly to the end of the kernel
                    for t4 in range(4):
                        cl = slice(t4 * 128, (t4 + 1) * 128)
                        for dc in range(4):
                            nc.scalar.copy(
                                at_sb[1][dc][:, cl], attn_ps[dc][:, cl])
                        emit_oproj_t4(3, t4, split_dma=(b == 1 and t4 == 3))
                else:
                    for dc in range(4):
                        nc.scalar.copy(at_sb[n % 2][dc][:], attn_ps[dc][:])

        def b1_tt0_v():
            # batch 1's first v projection fills the PE while batch 0's
            # final reciprocal/normalize chain drains; banks 0,1 (scores)
            # are the only ones not gated by that chain
            emit_v(0, xt_pre, banks=(0, 1))

        attn_phase(0, tail_filler=b1_tt0_v)

        # =========== batch 1 projection (v of tt0 already emitted) ===========
        emit_qk(0, xt_pre, cs_pre, sn_pre)
        for tt in range(1, 4):
            g0 = T + tt * 512
            s0 = tt * 512
            xt = xp.tile([128, 8, 512], f32r, tag="xt", name="xt")
            cs = csp.tile([128, 2, 512], f32, tag="cs", name="cs")
            sn = csp.tile([128, 2, 512], f32, tag="sn", name="sn")
            nc.sync.dma_start(xt[:], xT_r[:, :, g0:g0 + 512])
            nc.sync.dma_start(cs[:], cos_r[:, :, s0:s0 + 512])
            nc.sync.dma_start(sn[:], sin_r[:, :, s0:s0 + 512])
            if tt < 3:
                emit_v(tt, xt)
                emit_qk(s0, xt, cs, sn)
            else:
                emit_qk(s0, xt, cs, sn)
                emit_v(tt, xt)

        attn_phase(1)
    nc.compile()
    return nc


def _host_tables():
    import ml_dtypes
    inv_freq = 1.0 / (ROPE_BASE ** (np.arange(0, D, 2, dtype=np.float64) / D))
    ang = np.arange(T, dtype=np.float64)[:, None] * inv_freq[None, :]  # [T, D/2]
    cosdt = np.ascontiguousarray(np.cos(ang).T.astype(np.float32))     # [D/2, T]
    sindt = np.ascontiguousarray(np.sin(ang).T.astype(np.float32))
    jj = np.arange(128)
    kk = np.arange(128)
    qq = np.arange(512)
    a0 = np.where(jj[:, None] <= kk[None, :], NEG, 0.0).astype(ml_dtypes.bfloat16)
    bm = np.zeros((128, 4, 512), dtype=np.float32)
    for r in range(4):
        bm[:, r, :] = (jj[:, None] > (qq[None, :] - 128 * r)).astype(np.float32)
    return cosdt, sindt, a0, bm.astype(ml_dtypes.bfloat16)


def kernel(x, Wq, Wk, Wv, Wo):
    global LAST_RESULTS
    import ml_dtypes
    from concourse import bass_utils

    if "nc" not in _CACHE:
        _CACHE["nc"] = _build()
    nc = _CACHE["nc"]

    x = np.asarray(x, dtype=np.float32)
    Wq = np.asarray(Wq, dtype=np.float32)
    Wk = np.asarray(Wk, dtype=np.float32)
    Wv = np.asarray(Wv, dtype=np.float32)
    Wo = np.asarray(Wo, dtype=np.float32)

    xT = np.ascontiguousarray(x.reshape(NTOK, E).T)          # [E, NTOK]
    cosdt, sindt, a0, bm = _host_tables()

    in_maps = []
    for h in range(H):
        in_maps.append({
            "xT": xT,
            "wqT": np.ascontiguousarray(Wq[h * D:(h + 1) * D, :].T),
            "wkT": np.ascontiguousarray(Wk[h * D:(h + 1) * D, :].T),
            "wvT": np.ascontiguousarray(Wv[h * D:(h + 1) * D, :].T),
            "woT": np.ascontiguousarray(
                Wo[:, h * D:(h + 1) * D].T).astype(ml_dtypes.bfloat16),
            "cosdt": cosdt,
            "sindt": sindt,
            "a0t": a0,
            "bmt": bm,
        })

    kwargs = {}
    if PROFILE:
        import sys
        import types
        import trn_agent_boot.trn_boot as _tb
        hook = _tb._ntff_profile_via_ctypes("/opt/axon/libaxon_pjrt.so")
        mod = types.ModuleType("antenv.axon_hooks")
        mod.get_axon_ntff_profile_hook = lambda: hook
        mod.set_axon_ntff_profile_hook = lambda h_: None
        sys.modules["antenv.axon_hooks"] = mod
        bass_utils.upload_artifacts = lambda tmpdir: tmpdir
        kwargs = dict(trace=True, trace_cores=[0])

    res = bass_utils.run_bass_kernel_spmd(
        nc, in_maps, core_ids=list(range(H)), **kwargs)
    LAST_RESULTS = res

    out = res.results[0]["out"].astype(np.float32).copy()
    for h in range(1, H):
        out += res.results[h]["out"]
    return out.reshape(B, T, E)


# revision 56
# speedup vs baseline: 1.0011x; 1.0011x over previous
"""Trainium2 Bass kernel: multi-head attention (B=2, T=2048, E=1024, H=8, D=512),
bias-free QKV/O projections + RoPE + causal softmax.

Sharding: head-parallel across 8 NeuronCores. Core h computes head h fully:
  qT/kT = RoPE(Wq_h @ x.T), v = x @ Wv_h.T         (projection phase)
  scoresT[k,q] = kT.T @ qT   (per 512-wide q tile, causal-skipped k chunks;
    diagonal chunk r also skips its fully-masked first 128r columns)
  causal mask applied as a 5th accumulated matmul: (-C*[j<=kk]).T @ [j>qq-128r]
  gives -C*max(0, kk+128r-qq) in PSUM, so exp sees -inf off the triangle and
  the score->exp chain never leaves the PE/ACT engines.
  probsT = exp(scale*scoresT)                      (no max-subtraction: |s|<=9)
  attnT[d,q] = v.T @ probsT (unnormalized!); the softmax denominator is
  accumulated on the idle Pool engine (acc += exp chunk), reduced to a
  per-TOKEN column by 4 tiny transposed matmuls (acc_slice.T @ ones), and a
  [128,16] reciprocal; since 1/rowsum is a per-token scalar it commutes with
  o_proj and is fused into the o_proj PSUM evacuation as an ACT scale.
  out_h = attnT.T @ Wo_h.T * inv                   (partial o_proj, [4096,1024])
Host sums the 8 partial outputs (equivalent to the all-reduce after o_proj).

Matmuls run fp32r except PV + o_proj which run bf16 (probs/v/attn/Wo are
bf16: probs in [0,1] and attn magnitudes are tolerant; PSUM accumulates fp32).
fp32r moving dims stay >= 256 (ISA restriction; narrower runs 4 cyc/row).
PSUM is managed as 8 explicitly-tagged banks so phase-to-phase reuse pairs
earliest-freed banks with earliest-needed, keeping the PE stream gap-free:
startup DMAs are split per 256KB chunk with e-outer matmul ordering so the PE
chases the stream from ~10us; batch 1's first x tile is prefetched during
batch 0's attention and its v projection fills the PE while batch 0's final
rowsum chain drains; the previous q tile's o_proj is emitted one token tile
per chunk so its PSUM evacuations interleave with the exps on ACT.
Measured ~394-402us (baseline 446us), rel err ~3.2e-3.
"""
from contextlib import ExitStack

import numpy as np

B, T, E, H, D = 2, 2048, 1024, 8, 512
NTOK = B * T
SCALE = float(1.0 / np.sqrt(D))
NEG = -1.0e30
ROPE_BASE = 10000.0

PROFILE = False          # set True (e.g. from test.py) to trace core 0
LAST_RESULTS = None      # BassKernelResults of the last run when PROFILE

_CACHE = {}


def _build():
    import concourse.tile as tile
    from concourse import bacc, mybir

    f32 = mybir.dt.float32
    f32r = mybir.dt.float32r
    bf16 = mybir.dt.bfloat16
    AF = mybir.ActivationFunctionType

    nc = bacc.Bacc("TRN2", target_bir_lowering=False, debug=False,
                   enable_asserts=False, num_devices=8)
    xT_d = nc.dram_tensor("xT", [E, NTOK], f32r, kind="ExternalInput").ap()
    wqT_d = nc.dram_tensor("wqT", [E, D], f32r, kind="ExternalInput").ap()
    wkT_d = nc.dram_tensor("wkT", [E, D], f32r, kind="ExternalInput").ap()
    wvT_d = nc.dram_tensor("wvT", [E, D], f32r, kind="ExternalInput").ap()
    woT_d = nc.dram_tensor("woT", [D, E], bf16, kind="ExternalInput").ap()
    cos_d = nc.dram_tensor("cosdt", [D // 2, T], f32, kind="ExternalInput").ap()
    sin_d = nc.dram_tensor("sindt", [D // 2, T], f32, kind="ExternalInput").ap()
    a0_d = nc.dram_tensor("a0t", [128, 128], bf16, kind="ExternalInput").ap()
    bm_d = nc.dram_tensor("bmt", [128, 4, 512], bf16, kind="ExternalInput").ap()
    out_d = nc.dram_tensor("out", [NTOK, E], f32, kind="ExternalOutput").ap()

    xT_r = xT_d.rearrange("(eo p) t -> p eo t", p=128)     # [128, 8, 4096]
    cos_r = cos_d.rearrange("(fo p) t -> p fo t", p=128)   # [128, 2, 2048]
    sin_r = sin_d.rearrange("(fo p) t -> p fo t", p=128)
    wq_v = wqT_d.rearrange("(eo p) d -> p eo d", p=128)
    wk_v = wkT_d.rearrange("(eo p) d -> p eo d", p=128)
    wv_v = wvT_d.rearrange("(eo p) d -> p eo d", p=128)

    with tile.TileContext(nc) as tc, ExitStack() as top:
        # ---- all pools top-level: no mid-kernel pool boundaries ----
        wp = top.enter_context(tc.tile_pool(name="wp", bufs=1))
        qkvp = top.enter_context(tc.tile_pool(name="qkvp", bufs=1))
        xp = top.enter_context(tc.tile_pool(name="xp", bufs=2))
        csp = top.enter_context(tc.tile_pool(name="csp", bufs=1))
        scrp = top.enter_context(tc.tile_pool(name="scrp", bufs=2))
        epp = top.enter_context(tc.tile_pool(name="epp", bufs=4))
        atp = top.enter_context(tc.tile_pool(name="atp", bufs=1))
        accp = top.enter_context(tc.tile_pool(name="accp", bufs=1))
        ivp = top.enter_context(tc.tile_pool(name="ivp", bufs=1))
        pbp = top.enter_context(tc.tile_pool(name="pbp", bufs=1, space="PSUM"))

        def PB(k):
            return pbp.tile([128, 512], f32, tag=f"B{k}", name=f"B{k}")

        wq_t = wp.tile([128, 8, D], f32r, tag="wq", name="wq")
        wk_t = wp.tile([128, 8, D], f32r, tag="wk", name="wk")
        wv_t = wp.tile([128, 8, D], f32r, tag="wv", name="wv")
        wv = [wv_t[:, e] for e in range(8)]
        a0_t = wp.tile([128, 128], bf16, tag="a0", name="a0")
        bm_t = wp.tile([128, 4, 512], bf16, tag="bm", name="bm")
        ones = wp.tile([128, 128], f32r, tag="ones", name="ones")
        onesf = wp.tile([128, 4], f32, tag="onesf", name="onesf")
        expre = wp.tile([128, 1], f32, tag="expre", name="expre")
        wo_t = wp.tile([128, 4, E], bf16, tag="wo", name="wo")
        wo = [wo_t[:, d] for d in range(4)]

        qT = [qkvp.tile([128, T], f32r, tag=f"qT{d}", name=f"qT{d}") for d in range(4)]
        kT = [qkvp.tile([128, T], f32r, tag=f"kT{d}", name=f"kT{d}") for d in range(4)]
        vv = [qkvp.tile([128, D], bf16, tag=f"v{t}", name=f"v{t}") for t in range(16)]

        def rope(dstT, i, j, fo, pi, pj, cs, sn, s0):
            c_, s_ = cs[:, fo], sn[:, fo]
            sA = scrp.tile([128, 1024], f32, tag="scr", name="scr")
            sB = scrp.tile([128, 1024], f32, tag="scr", name="scr")
            t0, t1 = sA[:, 0:512], sA[:, 512:1024]
            t2, t3 = sB[:, 0:512], sB[:, 512:1024]
            nc.vector.tensor_mul(t0[:], pi[:], c_)
            nc.vector.tensor_mul(t1[:], pj[:], s_)
            nc.vector.tensor_sub(dstT[i][:, s0:s0 + 512], t0[:], t1[:])
            nc.vector.tensor_mul(t2[:], pi[:], s_)
            nc.vector.tensor_mul(t3[:], pj[:], c_)
            nc.vector.tensor_add(dstT[j][:, s0:s0 + 512], t2[:], t3[:])

        def emit_v(tt, xt, banks=(4, 5)):
            for t4 in range(4):
                ps_t = PB(banks[t4 % len(banks)])
                for e in range(8):
                    nc.tensor.matmul(
                        ps_t[:],
                        xt[:, e, t4 * 128:(t4 + 1) * 128],
                        wv[e][:],
                        start=(e == 0), stop=(e == 7))
                nc.scalar.copy(vv[tt * 4 + t4][:], ps_t[:])

        def emit_v_eouter(tt, xt, banks=(0, 1, 2, 3)):
            vps = [PB(banks[t4]) for t4 in range(4)]
            for e in range(8):
                for t4 in range(4):
                    nc.tensor.matmul(
                        vps[t4][:],
                        xt[:, e, t4 * 128:(t4 + 1) * 128],
                        wv[e][:],
                        start=(e == 0), stop=(e == 7))
            for t4 in range(4):
                nc.scalar.copy(vv[tt * 4 + t4][:], vps[t4][:])

        def emit_qk(s0, xt, cs, sn):
            rot = (6, 7, 0, 1, 2, 3)
            ri = 0
            for w_t, dstT in ((wq_t, qT), (wk_t, kT)):
                for i, j, fo in ((0, 2, 0), (1, 3, 1)):
                    ps2 = []
                    for dc in (i, j):
                        ps_t = PB(rot[ri % 6])
                        ri += 1
                        for e in range(8):
                            nc.tensor.matmul(
                                ps_t[:],
                                w_t[:, e, dc * 128:(dc + 1) * 128],
                                xt[:, e],
                                start=(e == 0), stop=(e == 7))
                        ps2.append(ps_t)
                    rope(dstT, i, j, fo, ps2[0], ps2[1], cs, sn, s0)

        def emit_qk_eouter(s0, xt, cs, sn):
            # all 4 dc of one projection accumulate e-outer so each matmul
            # needs only one freshly-DMA'd 256KB chunk pair
            for w_t, dstT, b0k in ((wq_t, qT, 4), (wk_t, kT, 0)):
                ps = {dc: PB(b0k + di) for di, dc in enumerate((0, 2, 1, 3))}
                for e in range(8):
                    for dc in (0, 2, 1, 3):
                        nc.tensor.matmul(
                            ps[dc][:],
                            w_t[:, e, dc * 128:(dc + 1) * 128],
                            xt[:, e],
                            start=(e == 0), stop=(e == 7))
                rope(dstT, 0, 2, 0, ps[0], ps[2], cs, sn, s0)
                rope(dstT, 1, 3, 1, ps[1], ps[3], cs, sn, s0)

        # =========== batch 0 projection ===========
        cs0 = {}
        for tt in range(4):
            g0 = tt * 512
            s0 = tt * 512
            xt = xp.tile([128, 8, 512], f32r, tag="xt", name="xt")
            cs = csp.tile([128, 2, 512], f32, tag="cs", name="cs")
            sn = csp.tile([128, 2, 512], f32, tag="sn", name="sn")
            if tt == 0:
                # need-ordered per-chunk DMAs: the PE chases the stream
                for e in range(8):
                    nc.sync.dma_start(xt[:, e], xT_r[:, e, g0:g0 + 512])
                    nc.sync.dma_start(wv_t[:, e], wv_v[:, e])
                for e in range(8):
                    nc.sync.dma_start(wq_t[:, e], wq_v[:, e])
                nc.sync.dma_start(cs[:], cos_r[:, :, s0:s0 + 512])
                nc.sync.dma_start(sn[:], sin_r[:, :, s0:s0 + 512])
                for e in range(8):
                    nc.sync.dma_start(wk_t[:, e], wk_v[:, e])
                # warmup: ramp the PE clock while the first DMAs stream in
                # (memset cannot target f32r: set f32 then cast-copy)
                onef = scrp.tile([128, 1024], f32, tag="scr", name="scr")
                nc.vector.memset(onef[:, :128], 1.0)
                nc.vector.tensor_copy(ones[:], onef[:, :128])
                nc.vector.memset(onesf[:], 1.0)
                warm_ps = PB(4)
                for w in range(12):
                    nc.tensor.matmul(warm_ps[:, :128], ones[:], ones[:],
                                     start=(w == 0), stop=(w == 11))
                nc.scalar.activation(expre[:], warm_ps[:, :1], AF.Exp,
                                     scale=0.001)
                nc.vector.tensor_copy(expre[:], expre[:])
                emit_v_eouter(tt, xt)
                emit_qk_eouter(s0, xt, cs, sn)
            elif tt == 1:
                for e in range(8):
                    nc.sync.dma_start(xt[:, e], xT_r[:, e, g0:g0 + 512])
                nc.sync.dma_start(cs[:], cos_r[:, :, s0:s0 + 512])
                nc.sync.dma_start(sn[:], sin_r[:, :, s0:s0 + 512])
                emit_v_eouter(tt, xt, banks=(4, 5, 6, 7))
                emit_qk(s0, xt, cs, sn)
            else:
                nc.sync.dma_start(xt[:], xT_r[:, :, g0:g0 + 512])
                nc.sync.dma_start(cs[:], cos_r[:, :, s0:s0 + 512])
                nc.sync.dma_start(sn[:], sin_r[:, :, s0:s0 + 512])
                if tt == 2:
                    emit_v(tt, xt)
                    emit_qk(s0, xt, cs, sn)
                else:
                    emit_qk(s0, xt, cs, sn)
                    emit_v(tt, xt)

        # prefetch batch 1's first x tile + rope tables; they transfer
        # during batch 0's attention so its projection starts stall-free
        xt_pre = xp.tile([128, 8, 512], f32r, tag="xt", name="xt")
        cs_pre = csp.tile([128, 2, 512], f32, tag="cs", name="cs")
        sn_pre = csp.tile([128, 2, 512], f32, tag="sn", name="sn")

        # =========== attention (both batches) ===========
        def attn_phase(b, tail_filler=None):
            tok0 = b * T
            SC = (0, 1, 2)
            ATB = (3, 4, 6, 7)
            if b == 0:
                nc.sync.dma_start(a0_t[:], a0_d)
                nc.sync.dma_start(bm_t[:], bm_d)
                nc.sync.dma_start(
                    wo_t[:], woT_d.rearrange("(do p) e -> p do e", p=128))
                nc.sync.dma_start(xt_pre[:], xT_r[:, :, T:T + 512])
                nc.sync.dma_start(cs_pre[:], cos_r[:, :, 0:512])
                nc.sync.dma_start(sn_pre[:], sin_r[:, :, 0:512])

            sci = [0]

            def emit_oproj_t4(n, t4, split_dma=False):
                q0 = n * 512
                r0 = tok0 + q0 + t4 * 128
                ob = scrp.tile([128, 1024], f32, tag="scr", name="scr")
                for et in range(2):
                    op_ps = PB(SC[sci[0] % 3])
                    sci[0] += 1
                    for dc in range(4):
                        nc.tensor.matmul(
                            op_ps[:],
                            at_sb[n % 2][dc][:, t4 * 128:(t4 + 1) * 128],
                            wo[dc][:, et * 512:(et + 1) * 512],
                            start=(dc == 0), stop=(dc == 3))
                    # attn was left unnormalized; the softmax 1/rowsum is
                    # a per-token scalar, so it commutes with o_proj and
                    # fuses into this evacuation copy for free
                    nc.scalar.activation(
                        ob[:, et * 512:(et + 1) * 512], op_ps[:],
                        AF.Copy, scale=inv_col[n % 2][:, t4 * 4:t4 * 4 + 1])
                    if split_dma:
                        nc.sync.dma_start(
                            out_d[r0:r0 + 128, et * 512:(et + 1) * 512],
                            ob[:, et * 512:(et + 1) * 512])
                if not split_dma:
                    nc.sync.dma_start(out_d[r0:r0 + 128, :], ob[:])

            at_sb = {0: None, 1: None}
            inv_col = {0: None, 1: None}
            for n in range(4):
                q0 = n * 512
                nch = 4 * n + 4
                attn_ps = [PB(ATB[d]) for d in range(4)]
                acc = accp.tile([128, 512], f32, tag="acc", name="acc")

                def emit_pv(pex, pc, nch=nch, attn_ps=attn_ps, n=n):
                    # columns [0, 128r) of a diagonal chunk are fully masked
                    # (exp == 0): skip them
                    s = max(0, (pc - 4 * n)) * 128
                    last = pc == nch - 1
                    for dc in range(4):
                        nc.tensor.matmul(
                            attn_ps[dc][:, s:512],
                            vv[pc][:, dc * 128:(dc + 1) * 128], pex[:, s:512],
                            start=(pc == 0), stop=last,
                            skip_group_check=(s > 0))

                pending = []
                for c in range(nch):
                    diag = c - 4 * n
                    sc_ps = PB(SC[sci[0] % 3])
                    sci[0] += 1
                    # diagonal chunk r: columns [0, 128r) are fully masked.
                    # scores (f32r) must stay >=256 wide; PV/exp/rowsum use
                    # the exact boundary s_pv = 128r
                    s_pv = max(0, diag) * 128
                    s_sc = min(s_pv, 256)
                    for dc in range(4):
                        nc.tensor.matmul(
                            sc_ps[:, s_sc:512],
                            kT[dc][:, c * 128:(c + 1) * 128],
                            qT[dc][:, q0 + s_sc:q0 + 512],
                            start=(dc == 0),
                            stop=(dc == 3))
                    if diag >= 0:
                        # causal mask as a 5th (narrow) accumulated matmul:
                        # -C * max(0, kk + 128*diag - qq) lands in PSUM; only
                        # the computed-and-maskable span [s_sc, 128(r+1)) or
                        # [s_sc, 512) for r == 3 needs it
                        w = 512 if diag == 3 else 128 * (diag + 1)
                        nc.tensor.matmul(
                            sc_ps[:, s_sc:w], a0_t[:], bm_t[:, diag, s_sc:w],
                            start=False, stop=True, skip_group_check=True)
                    ex = epp.tile([128, 512], bf16, tag="ex", name="ex")
                    nc.scalar.activation(
                        ex[:, s_sc:512], sc_ps[:, s_sc:512], AF.Exp, scale=SCALE)
                    # rowsum accumulates on the (otherwise idle) Pool engine
                    if c == 0:
                        nc.gpsimd.tensor_copy(acc[:], ex[:])
                    else:
                        nc.gpsimd.tensor_add(
                            acc[:, s_pv:512], acc[:, s_pv:512], ex[:, s_pv:512])
                    pending.append((ex, c))
                    if len(pending) > 3:
                        emit_pv(*pending.pop(0))
                    # previous group's o_proj, one token tile at a time so
                    # its PSUM evacuations interleave with the exps on ACT
                    if 2 <= c <= 5 and n > 0:
                        emit_oproj_t4(n - 1, c - 2)
                for pex, pc in pending:
                    emit_pv(pex, pc)
                # rowsum as a per-token COLUMN: 4 tiny transposed matmuls
                # (acc_slice.T @ ones[:, :1]); the reciprocal then runs on
                # just [128, 4] elements instead of [128, 512]
                rs_ps = PB(5)
                for t4 in range(4):
                    nc.tensor.matmul(
                        rs_ps[:, t4 * 4:(t4 + 1) * 4],
                        acc[:, t4 * 128:(t4 + 1) * 128], onesf[:, 0:4],
                        start=True, stop=True)
                inv_col[n % 2] = ivp.tile(
                    [128, 16], f32, tag=f"inv{n % 2}", name=f"inv{n % 2}")
                nc.vector.reciprocal(inv_col[n % 2][:], rs_ps[:, 0:16])
                # evacuate UNNORMALIZED via fast ACT copies: frees the PSUM
                # banks without waiting on any DVE chain
                at_sb[n % 2] = [
                    atp.tile([128, 512], bf16, tag=f"at{n % 2}_{dc}",
                             name=f"at{n % 2}_{dc}")
                    for dc in range(4)]
                if n == 3:
                    if tail_filler is not None:
                        tail_filler()
                    # final o_proj: evacuate each token tile's attn slice and
                    # immediately emit its o_proj so ACT copies, matmuls and
                    # output DMAs pipeline tightly to the end of the kernel
                    for t4 in range(4):
                        cl = slice(t4 * 128, (t4 + 1) * 128)
                        for dc in range(4):
                            nc.scalar.copy(
                                at_sb[1][dc][:, cl], attn_ps[dc][:, cl])
                        emit_oproj_t4(3, t4, split_dma=(b == 1 and t4 == 3))
                else:
                    for dc in range(4):
                        nc.scalar.copy(at_sb[n % 2][dc][:], attn_ps[dc][:])

        def b1_tt0_v():
            # batch 1's first v projection fills the PE while batch 0's
            # final reciprocal/normalize chain drains; banks 0,1 (scores)
            # are the only ones not gated by that chain
            emit_v(0, xt_pre, banks=(0, 1))

        attn_phase(0, tail_filler=b1_tt0_v)

        # =========== batch 1 projection (v of tt0 already emitted) ===========
        emit_qk(0, xt_pre, cs_pre, sn_pre)
        for tt in range(1, 4):
            g0 = T + tt * 512
            s0 = tt * 512
            xt = xp.tile([128, 8, 512], f32r, tag="xt", name="xt")
            cs = csp.tile([128, 2, 512], f32, tag="cs", name="cs")
            sn = csp.tile([128, 2, 512], f32, tag="sn", name="sn")
            nc.sync.dma_start(xt[:], xT_r[:, :, g0:g0 + 512])
            nc.sync.dma_start(cs[:], cos_r[:, :, s0:s0 + 512])
            nc.sync.dma_start(sn[:], sin_r[:, :, s0:s0 + 512])
            if tt < 3:
                emit_v(tt, xt)
                emit_qk(s0, xt, cs, sn)
            else:
                emit_qk(s0, xt, cs, sn)
                emit_v(tt, xt)

        attn_phase(1)
    nc.compile()
    return nc


def _host_tables():
    import ml_dtypes
    inv_freq = 1.0 / (ROPE_BASE ** (np.arange(0, D, 2, dtype=np.float64) / D))
    ang = np.arange(T, dtype=np.float64)[:, None] * inv_freq[None, :]  # [T, D/2]
    cosdt = np.ascontiguousarray(np.cos(ang).T.astype(np.float32))     # [D/2, T]
    sindt = np.ascontiguousarray(np.sin(ang).T.astype(np.float32))
    jj = np.arange(128)
    kk = np.arange(128)
    qq = np.arange(512)
    a0 = np.where(jj[:, None] <= kk[None, :], NEG, 0.0).astype(ml_dtypes.bfloat16)
    bm = np.zeros((128, 4, 512), dtype=np.float32)
    for r in range(4):
        bm[:, r, :] = (jj[:, None] > (qq[None, :] - 128 * r)).astype(np.float32)
    return cosdt, sindt, a0, bm.astype(ml_dtypes.bfloat16)


def kernel(x, Wq, Wk, Wv, Wo):
    global LAST_RESULTS
    import ml_dtypes
    from concourse import bass_utils

    if "nc" not in _CACHE:
        _CACHE["nc"] = _build()
    nc = _CACHE["nc"]

    x = np.asarray(x, dtype=np.float32)
    Wq = np.asarray(Wq, dtype=np.float32)
    Wk = np.asarray(Wk, dtype=np.float32)
    Wv = np.asarray(Wv, dtype=np.float32)
    Wo = np.asarray(Wo, dtype=np.float32)

    xT = np.ascontiguousarray(x.reshape(NTOK, E).T)          # [E, NTOK]
    cosdt, sindt, a0, bm = _host_tables()

    in_maps = []
    for h in range(H):
        in_maps.append({
            "xT": xT,
            "wqT": np.ascontiguousarray(Wq[h * D:(h + 1) * D, :].T),
            "wkT": np.ascontiguousarray(Wk[h * D:(h + 1) * D, :].T),
            "wvT": np.ascontiguousarray(Wv[h * D:(h + 1) * D, :].T),
            "woT": np.ascontiguousarray(
                Wo[:, h * D:(h + 1) * D].T).astype(ml_dtypes.bfloat16),
            "cosdt": cosdt,
            "sindt": sindt,
            "a0t": a0,
            "bmt": bm,
        })

    kwargs = {}
    if PROFILE:
        import sys
        import types
        import trn_agent_boot.trn_boot as _tb
        hook = _tb._ntff_profile_via_ctypes("/opt/axon/libaxon_pjrt.so")
        mod = types.ModuleType("antenv.axon_hooks")
        mod.get_axon_ntff_profile_hook = lambda: hook
        mod.set_axon_ntff_profile_hook = lambda h_: None
        sys.modules["antenv.axon_hooks"] = mod
        bass_utils.upload_artifacts = lambda tmpdir: tmpdir
        kwargs = dict(trace=True, trace_cores=[0])

    res = bass_utils.run_bass_kernel_spmd(
        nc, in_maps, core_ids=list(range(H)), **kwargs)
    LAST_RESULTS = res

    out = res.results[0]["out"].astype(np.float32).copy()
    for h in range(1, H):
        out += res.results[h]["out"]
    return out.reshape(B, T, E)


# revision 58
# speedup vs baseline: 1.0044x; 1.0033x over previous
"""Trainium2 Bass kernel: multi-head attention (B=2, T=2048, E=1024, H=8, D=512),
bias-free QKV/O projections + RoPE + causal softmax.

Sharding: head-parallel across 8 NeuronCores. Core h computes head h fully:
  qT/kT = RoPE(Wq_h @ x.T), v = x @ Wv_h.T         (projection phase)
  scoresT[k,q] = kT.T @ qT   (per 512-wide q tile, causal-skipped k chunks;
    diagonal chunk r also skips its fully-masked first 128r columns)
  causal mask applied as a 5th accumulated matmul: (-C*[j<=kk]).T @ [j>qq-128r]
  gives -C*max(0, kk+128r-qq) in PSUM, so exp sees -inf off the triangle and
  the score->exp chain never leaves the PE/ACT engines.
  probsT = exp(scale*scoresT)                      (no max-subtraction: |s|<=9)
  attnT[d,q] = v.T @ probsT (unnormalized!); the softmax denominator is
  accumulated on the idle Pool engine (acc += exp chunk), reduced to a
  per-TOKEN column by 4 tiny transposed matmuls (acc_slice.T @ ones), and a
  [128,16] reciprocal; since 1/rowsum is a per-token scalar it commutes with
  o_proj and is fused into the o_proj PSUM evacuation as an ACT scale.
  out_h = attnT.T @ Wo_h.T * inv                   (partial o_proj, [4096,1024])
Host sums the 8 partial outputs (equivalent to the all-reduce after o_proj).

Matmuls run fp32r except PV + o_proj which run bf16 (probs/v/attn/Wo are
bf16: probs in [0,1] and attn magnitudes are tolerant; PSUM accumulates fp32).
fp32r moving dims stay >= 256 (ISA restriction; narrower runs 4 cyc/row).
PSUM is managed as 8 explicitly-tagged banks so phase-to-phase reuse pairs
earliest-freed banks with earliest-needed, keeping the PE stream gap-free:
startup DMAs are split per 256KB chunk with e-outer matmul ordering so the PE
chases the stream from ~10us; batch 1's first x tile is prefetched during
batch 0's attention and its v projection fills the PE while batch 0's final
rowsum chain drains; the previous q tile's o_proj is emitted one token tile
per chunk so its PSUM evacuations interleave with the exps on ACT.
Measured ~394-402us (baseline 446us), rel err ~3.2e-3.
"""
from contextlib import ExitStack

import numpy as np

B, T, E, H, D = 2, 2048, 1024, 8, 512
NTOK = B * T
SCALE = float(1.0 / np.sqrt(D))
NEG = -1.0e30
ROPE_BASE = 10000.0

PROFILE = False          # set True (e.g. from test.py) to trace core 0
LAST_RESULTS = None      # BassKernelResults of the last run when PROFILE

_CACHE = {}


def _build():
    import concourse.tile as tile
    from concourse import bacc, mybir

    f32 = mybir.dt.float32
    f32r = mybir.dt.float32r
    bf16 = mybir.dt.bfloat16
    AF = mybir.ActivationFunctionType

    nc = bacc.Bacc("TRN2", target_bir_lowering=False, debug=False,
                   enable_asserts=False, num_devices=8)
    xT_d = nc.dram_tensor("xT", [E, NTOK], f32r, kind="ExternalInput").ap()
    wqT_d = nc.dram_tensor("wqT", [E, D], f32r, kind="ExternalInput").ap()
    wkT_d = nc.dram_tensor("wkT", [E, D], f32r, kind="ExternalInput").ap()
    wvT_d = nc.dram_tensor("wvT", [E, D], f32r, kind="ExternalInput").ap()
    woT_d = nc.dram_tensor("woT", [D, E], bf16, kind="ExternalInput").ap()
    cos_d = nc.dram_tensor("cosdt", [D // 2, T], f32, kind="ExternalInput").ap()
    sin_d = nc.dram_tensor("sindt", [D // 2, T], f32, kind="ExternalInput").ap()
    a0_d = nc.dram_tensor("a0t", [128, 128], bf16, kind="ExternalInput").ap()
    bm_d = nc.dram_tensor("bmt", [128, 4, 512], bf16, kind="ExternalInput").ap()
    out_d = nc.dram_tensor("out", [NTOK, E], f32, kind="ExternalOutput").ap()

    xT_r = xT_d.rearrange("(eo p) t -> p eo t", p=128)     # [128, 8, 4096]
    cos_r = cos_d.rearrange("(fo p) t -> p fo t", p=128)   # [128, 2, 2048]
    sin_r = sin_d.rearrange("(fo p) t -> p fo t", p=128)
    wq_v = wqT_d.rearrange("(eo p) d -> p eo d", p=128)
    wk_v = wkT_d.rearrange("(eo p) d -> p eo d", p=128)
    wv_v = wvT_d.rearrange("(eo p) d -> p eo d", p=128)

    with tile.TileContext(nc) as tc, ExitStack() as top:
        # ---- all pools top-level: no mid-kernel pool boundaries ----
        wp = top.enter_context(tc.tile_pool(name="wp", bufs=1))
        qkvp = top.enter_context(tc.tile_pool(name="qkvp", bufs=1))
        xp = top.enter_context(tc.tile_pool(name="xp", bufs=2))
        csp = top.enter_context(tc.tile_pool(name="csp", bufs=1))
        scrp = top.enter_context(tc.tile_pool(name="scrp", bufs=2))
        epp = top.enter_context(tc.tile_pool(name="epp", bufs=4))
        atp = top.enter_context(tc.tile_pool(name="atp", bufs=1))
        accp = top.enter_context(tc.tile_pool(name="accp", bufs=1))
        ivp = top.enter_context(tc.tile_pool(name="ivp", bufs=1))
        pbp = top.enter_context(tc.tile_pool(name="pbp", bufs=1, space="PSUM"))

        def PB(k):
            return pbp.tile([128, 512], f32, tag=f"B{k}", name=f"B{k}")

        wq_t = wp.tile([128, 8, D], f32r, tag="wq", name="wq")
        wk_t = wp.tile([128, 8, D], f32r, tag="wk", name="wk")
        wv_t = wp.tile([128, 8, D], f32r, tag="wv", name="wv")
        wv = [wv_t[:, e] for e in range(8)]
        a0_t = wp.tile([128, 128], bf16, tag="a0", name="a0")
        bm_t = wp.tile([128, 4, 512], bf16, tag="bm", name="bm")
        ones = wp.tile([128, 128], f32r, tag="ones", name="ones")
        onesf = wp.tile([128, 4], f32, tag="onesf", name="onesf")
        expre = wp.tile([128, 1], f32, tag="expre", name="expre")
        wo_t = wp.tile([128, 4, E], bf16, tag="wo", name="wo")
        wo = [wo_t[:, d] for d in range(4)]

        qT = [qkvp.tile([128, T], f32r, tag=f"qT{d}", name=f"qT{d}") for d in range(4)]
        kT = [qkvp.tile([128, T], f32r, tag=f"kT{d}", name=f"kT{d}") for d in range(4)]
        vv = [qkvp.tile([128, D], bf16, tag=f"v{t}", name=f"v{t}") for t in range(16)]

        def rope(dstT, i, j, fo, pi, pj, cs, sn, s0):
            c_, s_ = cs[:, fo], sn[:, fo]
            sA = scrp.tile([128, 1024], f32, tag="scr", name="scr")
            sB = scrp.tile([128, 1024], f32, tag="scr", name="scr")
            t0, t1 = sA[:, 0:512], sA[:, 512:1024]
            t2, t3 = sB[:, 0:512], sB[:, 512:1024]
            nc.vector.tensor_mul(t0[:], pi[:], c_)
            nc.vector.tensor_mul(t1[:], pj[:], s_)
            nc.vector.tensor_sub(dstT[i][:, s0:s0 + 512], t0[:], t1[:])
            nc.vector.tensor_mul(t2[:], pi[:], s_)
            nc.vector.tensor_mul(t3[:], pj[:], c_)
            nc.vector.tensor_add(dstT[j][:, s0:s0 + 512], t2[:], t3[:])

        def emit_v(tt, xt, banks=(4, 5)):
            for t4 in range(4):
                ps_t = PB(banks[t4 % len(banks)])
                for e in range(8):
                    nc.tensor.matmul(
                        ps_t[:],
                        xt[:, e, t4 * 128:(t4 + 1) * 128],
                        wv[e][:],
                        start=(e == 0), stop=(e == 7))
                nc.scalar.copy(vv[tt * 4 + t4][:], ps_t[:])

        def emit_v_eouter(tt, xt, banks=(0, 1, 2, 3)):
            vps = [PB(banks[t4]) for t4 in range(4)]
            for e in range(8):
                for t4 in range(4):
                    nc.tensor.matmul(
                        vps[t4][:],
                        xt[:, e, t4 * 128:(t4 + 1) * 128],
                        wv[e][:],
                        start=(e == 0), stop=(e == 7))
            for t4 in range(4):
                nc.scalar.copy(vv[tt * 4 + t4][:], vps[t4][:])

        def emit_qk(s0, xt, cs, sn):
            rot = (6, 7, 0, 1, 2, 3)
            ri = 0
            for w_t, dstT in ((wq_t, qT), (wk_t, kT)):
                for i, j, fo in ((0, 2, 0), (1, 3, 1)):
                    ps2 = []
                    for dc in (i, j):
                        ps_t = PB(rot[ri % 6])
                        ri += 1
                        for e in range(8):
                            nc.tensor.matmul(
                                ps_t[:],
                                w_t[:, e, dc * 128:(dc + 1) * 128],
                                xt[:, e],
                                start=(e == 0), stop=(e == 7))
                        ps2.append(ps_t)
                    rope(dstT, i, j, fo, ps2[0], ps2[1], cs, sn, s0)

        def emit_qk_eouter(s0, xt, cs, sn):
            # all 4 dc of one projection accumulate e-outer so each matmul
            # needs only one freshly-DMA'd 256KB chunk pair
            for w_t, dstT, b0k in ((wq_t, qT, 4), (wk_t, kT, 0)):
                ps = {dc: PB(b0k + di) for di, dc in enumerate((0, 2, 1, 3))}
                for e in range(8):
                    for dc in (0, 2, 1, 3):
                        nc.tensor.matmul(
                            ps[dc][:],
                            w_t[:, e, dc * 128:(dc + 1) * 128],
                            xt[:, e],
                            start=(e == 0), stop=(e == 7))
                rope(dstT, 0, 2, 0, ps[0], ps[2], cs, sn, s0)
                rope(dstT, 1, 3, 1, ps[1], ps[3], cs, sn, s0)

        # =========== batch 0 projection ===========
        cs0 = {}
        for tt in range(4):
            g0 = tt * 512
            s0 = tt * 512
            xt = xp.tile([128, 8, 512], f32r, tag="xt", name="xt")
            cs = csp.tile([128, 2, 512], f32, tag="cs", name="cs")
            sn = csp.tile([128, 2, 512], f32, tag="sn", name="sn")
            if tt == 0:
                # need-ordered per-chunk DMAs: the PE chases the stream
                for e in range(8):
                    nc.sync.dma_start(xt[:, e], xT_r[:, e, g0:g0 + 512])
                    nc.sync.dma_start(wv_t[:, e], wv_v[:, e])
                for e in range(8):
                    nc.sync.dma_start(wq_t[:, e], wq_v[:, e])
                nc.sync.dma_start(cs[:], cos_r[:, :, s0:s0 + 512])
                nc.sync.dma_start(sn[:], sin_r[:, :, s0:s0 + 512])
                for e in range(8):
                    nc.sync.dma_start(wk_t[:, e], wk_v[:, e])
                # warmup: ramp the PE clock while the first DMAs stream in
                # (memset cannot target f32r: set f32 then cast-copy)
                onef = scrp.tile([128, 1024], f32, tag="scr", name="scr")
                nc.vector.memset(onef[:, :128], 1.0)
                nc.vector.tensor_copy(ones[:], onef[:, :128])
                nc.vector.memset(onesf[:], 1.0)
                warm_ps = PB(4)
                for w in range(12):
                    nc.tensor.matmul(warm_ps[:, :128], ones[:], ones[:],
                                     start=(w == 0), stop=(w == 11))
                nc.scalar.activation(expre[:], warm_ps[:, :1], AF.Exp,
                                     scale=0.001)
                nc.vector.tensor_copy(expre[:], expre[:])
                emit_v_eouter(tt, xt)
                emit_qk_eouter(s0, xt, cs, sn)
            elif tt == 1:
                for e in range(8):
                    nc.sync.dma_start(xt[:, e], xT_r[:, e, g0:g0 + 512])
                nc.sync.dma_start(cs[:], cos_r[:, :, s0:s0 + 512])
                nc.sync.dma_start(sn[:], sin_r[:, :, s0:s0 + 512])
                emit_v_eouter(tt, xt, banks=(4, 5, 6, 7))
                emit_qk(s0, xt, cs, sn)
            else:
                nc.sync.dma_start(xt[:], xT_r[:, :, g0:g0 + 512])
                nc.sync.dma_start(cs[:], cos_r[:, :, s0:s0 + 512])
                nc.sync.dma_start(sn[:], sin_r[:, :, s0:s0 + 512])
                if tt == 2:
                    emit_v(tt, xt)
                    emit_qk(s0, xt, cs, sn)
                else:
                    emit_qk(s0, xt, cs, sn)
                    emit_v(tt, xt)

        # prefetch batch 1's first x tile + rope tables; they transfer
        # during batch 0's attention so its projection starts stall-free
        xt_pre = xp.tile([128, 8, 512], f32r, tag="xt", name="xt")
        cs_pre = csp.tile([128, 2, 512], f32, tag="cs", name="cs")
        sn_pre = csp.tile([128, 2, 512], f32, tag="sn", name="sn")

        # =========== attention (both batches) ===========
        def attn_phase(b, tail_filler=None):
            tok0 = b * T
            SC = (0, 1, 2)
            ATB = (3, 4, 6, 7)
            if b == 0:
                nc.sync.dma_start(a0_t[:], a0_d)
                nc.sync.dma_start(bm_t[:], bm_d)
                nc.sync.dma_start(
                    wo_t[:], woT_d.rearrange("(do p) e -> p do e", p=128))
                nc.sync.dma_start(xt_pre[:], xT_r[:, :, T:T + 512])
                nc.sync.dma_start(cs_pre[:], cos_r[:, :, 0:512])
                nc.sync.dma_start(sn_pre[:], sin_r[:, :, 0:512])

            sci = [0]

            def emit_oproj_t4(n, t4, split_dma=False):
                q0 = n * 512
                r0 = tok0 + q0 + t4 * 128
                ob = scrp.tile([128, 1024], f32, tag="scr", name="scr")
                for et in range(2):
                    op_ps = PB(SC[sci[0] % 3])
                    sci[0] += 1
                    for dc in range(4):
                        nc.tensor.matmul(
                            op_ps[:],
                            at_sb[n % 2][dc][:, t4 * 128:(t4 + 1) * 128],
                            wo[dc][:, et * 512:(et + 1) * 512],
                            start=(dc == 0), stop=(dc == 3))
                    # attn was left unnormalized; the softmax 1/rowsum is
                    # a per-token scalar, so it commutes with o_proj and
                    # fuses into this evacuation copy for free
                    nc.scalar.activation(
                        ob[:, et * 512:(et + 1) * 512], op_ps[:],
                        AF.Copy, scale=inv_col[n % 2][:, t4 * 4:t4 * 4 + 1])
                    if split_dma:
                        nc.sync.dma_start(
                            out_d[r0:r0 + 128, et * 512:(et + 1) * 512],
                            ob[:, et * 512:(et + 1) * 512])
                if not split_dma:
                    nc.sync.dma_start(out_d[r0:r0 + 128, :], ob[:])

            at_sb = {0: None, 1: None}
            inv_col = {0: None, 1: None}
            for n in range(4):
                q0 = n * 512
                nch = 4 * n + 4
                attn_ps = [PB(ATB[d]) for d in range(4)]
                acc = accp.tile([128, 512], f32, tag="acc", name="acc")

                def emit_pv(pex, pc, nch=nch, attn_ps=attn_ps, n=n):
                    # columns [0, 128r) of a diagonal chunk are fully masked
                    # (exp == 0): skip them
                    s = max(0, (pc - 4 * n)) * 128
                    last = pc == nch - 1
                    for dc in range(4):
                        nc.tensor.matmul(
                            attn_ps[dc][:, s:512],
                            vv[pc][:, dc * 128:(dc + 1) * 128], pex[:, s:512],
                            start=(pc == 0), stop=last,
                            skip_group_check=(s > 0))

                pending = []
                for c in range(nch):
                    diag = c - 4 * n
                    sc_ps = PB(SC[sci[0] % 3])
                    sci[0] += 1
                    # diagonal chunk r: columns [0, 128r) are fully masked.
                    # scores (f32r) must stay >=256 wide; PV/exp/rowsum use
                    # the exact boundary s_pv = 128r
                    s_pv = max(0, diag) * 128
                    s_sc = min(s_pv, 256)
                    for dc in range(4):
                        nc.tensor.matmul(
                            sc_ps[:, s_sc:512],
                            kT[dc][:, c * 128:(c + 1) * 128],
                            qT[dc][:, q0 + s_sc:q0 + 512],
                            start=(dc == 0),
                            stop=(dc == 3))
                    if diag >= 0:
                        # causal mask as a 5th (narrow) accumulated matmul:
                        # -C * max(0, kk + 128*diag - qq) lands in PSUM; only
                        # the computed-and-maskable span [s_sc, 128(r+1)) or
                        # [s_sc, 512) for r == 3 needs it
                        w = 512 if diag == 3 else 128 * (diag + 1)
                        nc.tensor.matmul(
                            sc_ps[:, s_sc:w], a0_t[:], bm_t[:, diag, s_sc:w],
                            start=False, stop=True, skip_group_check=True)
                    ex = epp.tile([128, 512], bf16, tag="ex", name="ex")
                    nc.scalar.activation(
                        ex[:, s_sc:512], sc_ps[:, s_sc:512], AF.Exp, scale=SCALE)
                    # rowsum accumulates on the (otherwise idle) Pool engine
                    if c == 0:
                        nc.gpsimd.tensor_copy(acc[:], ex[:])
                    else:
                        nc.gpsimd.tensor_add(
                            acc[:, s_pv:512], acc[:, s_pv:512], ex[:, s_pv:512])
                    pending.append((ex, c))
                    if len(pending) > 3:
                        emit_pv(*pending.pop(0))
                    # previous group's o_proj, one token tile at a time so
                    # its PSUM evacuations interleave with the exps on ACT
                    if 2 <= c <= 5 and n > 0:
                        emit_oproj_t4(n - 1, c - 2)
                # rowsum as a per-token COLUMN: 4 tiny transposed matmuls
                # (acc_slice.T @ ones[:, :4]); the reciprocal then runs on
                # just [128, 16] elements instead of [128, 512]. Diagonal
                # chunk r is the last Pool-writer of acc[:, 128r:128(r+1)],
                # so each tiny matmul interleaves right after that chunk's
                # PV in the flush instead of serializing on the full chain.
                rs_ps = PB(5)
                rs_done = set()

                def emit_rs(t4):
                    rs_done.add(t4)
                    nc.tensor.matmul(
                        rs_ps[:, t4 * 4:(t4 + 1) * 4],
                        acc[:, t4 * 128:(t4 + 1) * 128], onesf[:, 0:4],
                        start=True, stop=True)

                for pex, pc in pending:
                    emit_pv(pex, pc)
                    for t4 in range(min(4, pc - 4 * n + 1)):
                        if t4 not in rs_done:
                            emit_rs(t4)
                for t4 in range(4):
                    if t4 not in rs_done:
                        emit_rs(t4)
                inv_col[n % 2] = ivp.tile(
                    [128, 16], f32, tag=f"inv{n % 2}", name=f"inv{n % 2}")
                nc.vector.reciprocal(inv_col[n % 2][:], rs_ps[:, 0:16])
                # evacuate UNNORMALIZED via fast ACT copies: frees the PSUM
                # banks without waiting on any DVE chain
                at_sb[n % 2] = [
                    atp.tile([128, 512], bf16, tag=f"at{n % 2}_{dc}",
                             name=f"at{n % 2}_{dc}")
                    for dc in range(4)]
                if n == 3:
                    if tail_filler is not None:
                        tail_filler()
                    # final o_proj: evacuate each token tile's attn slice and
                    # immediately emit its o_proj so ACT copies, matmuls and
                    # output DMAs pipeline tightly to the end of the kernel
                    for t4 in range(4):
                        cl = slice(t4 * 128, (t4 + 1) * 128)
                        for dc in range(4):
                            nc.scalar.copy(
                                at_sb[1][dc][:, cl], attn_ps[dc][:, cl])
                        emit_oproj_t4(3, t4, split_dma=(b == 1 and t4 == 3))
                else:
                    for dc in range(4):
                        nc.scalar.copy(at_sb[n % 2][dc][:], attn_ps[dc][:])

        def b1_tt0_v():
            # batch 1's first v projection fills the PE while batch 0's
            # final reciprocal/normalize chain drains; banks 0,1 (scores)
            # are the only ones not gated by that chain
            emit_v(0, xt_pre, banks=(0, 1))

        attn_phase(0, tail_filler=b1_tt0_v)

        # =========== batch 1 projection (v of tt0 already emitted) ===========
        emit_qk(0, xt_pre, cs_pre, sn_pre)
        for tt in range(1, 4):
            g0 = T + tt * 512
            s0 = tt * 512
            xt = xp.tile([128, 8, 512], f32r, tag="xt", name="xt")
            cs = csp.tile([128, 2, 512], f32, tag="cs", name="cs")
            sn = csp.tile([128, 2, 512], f32, tag="sn", name="sn")
            nc.sync.dma_start(xt[:], xT_r[:, :, g0:g0 + 512])
            nc.sync.dma_start(cs[:], cos_r[:, :, s0:s0 + 512])
            nc.sync.dma_start(sn[:], sin_r[:, :, s0:s0 + 512])
            if tt < 3:
                emit_v(tt, xt)
                emit_qk(s0, xt, cs, sn)
            else:
                emit_qk(s0, xt, cs, sn)
                emit_v(tt, xt)

        attn_phase(1)
    nc.compile()
    return nc


def _host_tables():
    import ml_dtypes
    inv_freq = 1.0 / (ROPE_BASE ** (np.arange(0, D, 2, dtype=np.float64) / D))
    ang = np.arange(T, dtype=np.float64)[:, None] * inv_freq[None, :]  # [T, D/2]
    cosdt = np.ascontiguousarray(np.cos(ang).T.astype(np.float32))     # [D/2, T]
    sindt = np.ascontiguousarray(np.sin(ang).T.astype(np.float32))
    jj = np.arange(128)
    kk = np.arange(128)
    qq = np.arange(512)
    a0 = np.where(jj[:, None] <= kk[None, :], NEG, 0.0).astype(ml_dtypes.bfloat16)
    bm = np.zeros((128, 4, 512), dtype=np.float32)
    for r in range(4):
        bm[:, r, :] = (jj[:, None] > (qq[None, :] - 128 * r)).astype(np.float32)
    return cosdt, sindt, a0, bm.astype(ml_dtypes.bfloat16)


def kernel(x, Wq, Wk, Wv, Wo):
    global LAST_RESULTS
    import ml_dtypes
    from concourse import bass_utils

    if "nc" not in _CACHE:
        _CACHE["nc"] = _build()
    nc = _CACHE["nc"]

    x = np.asarray(x, dtype=np.float32)
    Wq = np.asarray(Wq, dtype=np.float32)
    Wk = np.asarray(Wk, dtype=np.float32)
    Wv = np.asarray(Wv, dtype=np.float32)
    Wo = np.asarray(Wo, dtype=np.float32)

    xT = np.ascontiguousarray(x.reshape(NTOK, E).T)          # [E, NTOK]
    cosdt, sindt, a0, bm = _host_tables()

    in_maps = []
    for h in range(H):
        in_maps.append({
            "xT": xT,
            "wqT": np.ascontiguousarray(Wq[h * D:(h + 1) * D, :].T),
            "wkT": np.ascontiguousarray(Wk[h * D:(h + 1) * D, :].T),
            "wvT": np.ascontiguousarray(Wv[h * D:(h + 1) * D, :].T),
            "woT": np.ascontiguousarray(
                Wo[:, h * D:(h + 1) * D].T).astype(ml_dtypes.bfloat16),
            "cosdt": cosdt,
            "sindt": sindt,
            "a0t": a0,
            "bmt": bm,
        })

    kwargs = {}
    if PROFILE:
        import sys
        import types
        import trn_agent_boot.trn_boot as _tb
        hook = _tb._ntff_profile_via_ctypes("/opt/axon/libaxon_pjrt.so")
        mod = types.ModuleType("antenv.axon_hooks")
        mod.get_axon_ntff_profile_hook = lambda: hook
        mod.set_axon_ntff_profile_hook = lambda h_: None
        sys.modules["antenv.axon_hooks"] = mod
        bass_utils.upload_artifacts = lambda tmpdir: tmpdir
        kwargs = dict(trace=True, trace_cores=[0])

    res = bass_utils.run_bass_kernel_spmd(
        nc, in_maps, core_ids=list(range(H)), **kwargs)
    LAST_RESULTS = res

    out = res.results[0]["out"].astype(np.float32).copy()
    for h in range(1, H):
        out += res.results[h]["out"]
    return out.reshape(B, T, E)
